# revision 6
# baseline (speedup 1.0000x reference)
"""GraphSAGE-mean 2-layer GNN kernel for 8 Trainium2 NeuronCores.

Strategy: shard dst nodes across 8 cores. Host does *index* preprocessing
only (sort edges by dst, sort nodes by degree, pad per-node edge lists to a
per-chunk common degree). The device does all FLOPs + all data movement of
feature payloads:
  - layer1: indirect-DMA gather feat[src] rows -> DVE strided segmented
    reduce -> mean -> PE matmuls (self+neigh) + ReLU -> h^T
  - p = h @ W2_neigh per core, AllGather p across the 8 cores (on-chip)
  - layer2: gather p[src] -> reduce -> + self term (bias folded in via an
    augmented ones-row of h) -> log_softmax -> out
"""

import os
import sys

sys.path.insert(0, "/opt/trn_rl_repo")

import numpy as np

import concourse.bacc as bacc
import concourse.bass as bass
import concourse.tile as tile
from concourse import mybir
from concourse.bass_utils import run_bass_kernel_spmd
from concourse.masks import make_identity

F32 = mybir.dt.float32
I32 = mybir.dt.int32

NCORES = 8
P = 128

# exposed for test.py: results object of the last run (exec_time_ns etc.)
LAST_RESULTS = None
LAST_NC = None


# --------------------------------------------------------------------------
# host-side index preprocessing
# --------------------------------------------------------------------------
def _prep_indices(src, dst, n_nodes, ncores):
    """Build per-core padded gather-index tables.

    Returns dict with:
      D        [NCH] common padded degree per chunk (over all cores)
      off      [NCH+1] col offsets, sumD = off[-1]
      idx1     [ncores, 128, sumD] int32 indices into feat_aug rows (dummy=n_nodes)
      idx2     [ncores, 128, sumD] int32 indices into p_full rows (dummy=ncores*npad)
      perms    list of per-core node permutation (local ids, rank order)
      npad     padded nodes per core (multiple of 128)
    """
    E = src.shape[0]
    npc = n_nodes // ncores
    nch = (npc + P - 1) // P
    npad = nch * P

    core_of = dst // npc
    order = np.argsort(dst, kind="stable")
    dst_s = dst[order]
    src_s = src[order]
    core_s = core_of[order]

    deg_full = np.bincount(dst, minlength=n_nodes).astype(np.int64)

    pos = np.empty(n_nodes, np.int64)
    perms = []
    Ds = np.zeros((ncores, nch), np.int64)
    for c in range(ncores):
        degc = deg_full[c * npc : (c + 1) * npc]
        permc = np.argsort(-degc, kind="stable")
        perms.append(permc)
        dsort = degc[permc]
        dpad = np.zeros(npad, np.int64)
        dpad[:npc] = dsort
        Ds[c] = dpad.reshape(nch, P).max(axis=1)
        rankc = np.empty(npc, np.int64)
        rankc[permc] = np.arange(npc)
        pos[c * npc : (c + 1) * npc] = c * npad + rankc

    D = Ds.max(axis=0)
    off = np.zeros(nch + 1, np.int64)
    off[1:] = np.cumsum(D)
    sumD = int(off[-1])

    dummy1 = n_nodes
    dummy2 = ncores * npad
    idx1 = np.full((ncores, P, sumD), dummy1, np.int32)
    idx2 = np.full((ncores, P, sumD), dummy2, np.int32)

    # per-edge slot within its node (edges of one node are contiguous in dst_s)
    starts = np.zeros(n_nodes + 1, np.int64)
    starts[1:] = np.cumsum(np.bincount(dst_s, minlength=n_nodes))
    j_s = np.arange(E, dtype=np.int64) - starts[dst_s]

    r_s = pos[dst_s] - core_s * npad  # local rank
    k_s = r_s // P
    p_s = r_s % P
    col_s = off[k_s] + j_s

    idx1[core_s, p_s, col_s] = src_s.astype(np.int32)
    idx2[core_s, p_s, col_s] = pos[src_s].astype(np.int32)

    return dict(D=D, off=off, sumD=sumD, idx1=idx1, idx2=idx2, perms=perms,
                npad=npad, nch=nch, npc=npc, pos=pos)


def _make_groups(D, off, dgmax=384, max_chunks=24):
    """Greedy-pack chunks into gather groups with sum(D) <= dgmax."""
    groups = []  # (k0, nk, colstart, dg)
    k0 = 0
    nch = len(D)
    while k0 < nch:
        dg = 0
        nk = 0
        while (k0 + nk) < nch and nk < max_chunks:
            dk = int(D[k0 + nk])
            if nk > 0 and dg + dk > dgmax:
                break
            dg += dk
            nk += 1
        groups.append((k0, nk, int(off[k0]), dg))
        k0 += nk
    return groups


# --------------------------------------------------------------------------
# device program
# --------------------------------------------------------------------------
def _build_program(meta, groups, f_in, f_hid, f_out, n_nodes, ncores):
    """Build the bass program (same for all cores)."""
    D = meta["D"]
    off = meta["off"]
    sumD = meta["sumD"]
    npad = meta["npad"]
    nch = meta["nch"]
    fh = f_hid + 1  # augmented hidden dim (ones row)
    dummy1 = n_nodes

    dg_tile = max((g[3] for g in groups), default=1)
    dg_tile = max(dg_tile, 1)
    nk_tile = max(g[1] for g in groups)

    nc = bacc.Bacc("TRN2", target_bir_lowering=False, debug=False,
                   num_devices=ncores)

    feat_aug = nc.dram_tensor("feat_aug", [n_nodes + 1, f_in], F32,
                              kind="ExternalInput")
    featT = nc.dram_tensor("featT", [f_in, npad], F32, kind="ExternalInput")
    idx1_d = nc.dram_tensor("idx1", [P, sumD], I32, kind="ExternalInput")
    idx2_d = nc.dram_tensor("idx2", [P, sumD], I32, kind="ExternalInput")
    w1s_d = nc.dram_tensor("w1s", [f_in, fh], F32, kind="ExternalInput")
    w1n_d = nc.dram_tensor("w1n", [f_in, fh], F32, kind="ExternalInput")
    b1_d = nc.dram_tensor("b1a", [fh, 1], F32, kind="ExternalInput")
    w2s_d = nc.dram_tensor("w2s", [fh, f_out], F32, kind="ExternalInput")
    w2n_d = nc.dram_tensor("w2n", [fh, f_out], F32, kind="ExternalInput")

    out_d = nc.dram_tensor("out_blk", [npad, f_out], F32, kind="ExternalOutput")

    hT_dram = nc.dram_tensor("hT_dram", [fh, npad], F32)
    p_blk = nc.dram_tensor("p_blk", [npad, f_out], F32)
    p_full = nc.dram_tensor("p_full", [ncores * npad + 1, f_out], F32,
                            addr_space="Shared")

    with tile.TileContext(nc) as tc:
        with (
            tc.tile_pool(name="const", bufs=1) as cpool,
            tc.tile_pool(name="idx", bufs=1) as ipool,
            tc.tile_pool(name="deg", bufs=1) as dpool,
            tc.tile_pool(name="mask", bufs=2) as mpool,
            tc.tile_pool(name="gather", bufs=2) as gpool,
            tc.tile_pool(name="stream", bufs=2) as spool,
            tc.tile_pool(name="work", bufs=3) as wpool,
            tc.tile_pool(name="small", bufs=4) as smpool,
            tc.tile_pool(name="psA", bufs=2, space="PSUM") as psA,
            tc.tile_pool(name="psB", bufs=2, space="PSUM") as psB,
        ):
            # ---- constants
            ident = cpool.tile([P, P], F32, tag="ident")
            make_identity(nc, ident[:])
            w1s = cpool.tile([f_in, fh], F32, tag="w1s")
            nc.sync.dma_start(out=w1s[:], in_=w1s_d[:])
            w1n = cpool.tile([f_in, fh], F32, tag="w1n")
            nc.sync.dma_start(out=w1n[:], in_=w1n_d[:])
            b1 = cpool.tile([fh, 1], F32, tag="b1")
            nc.sync.dma_start(out=b1[:], in_=b1_d[:])
            w2s = cpool.tile([fh, f_out], F32, tag="w2s")
            nc.sync.dma_start(out=w2s[:], in_=w2s_d[:])
            w2n = cpool.tile([fh, f_out], F32, tag="w2n")
            nc.sync.dma_start(out=w2n[:], in_=w2n_d[:])
            zrow = cpool.tile([1, f_out], F32, tag="zrow")
            nc.vector.memset(zrow[:], 0.0)

            # ---- index tables
            idx1 = ipool.tile([P, sumD], I32, tag="idx1")
            nc.sync.dma_start(out=idx1[:], in_=idx1_d[:])
            idx2 = ipool.tile([P, sumD], I32, tag="idx2")
            nc.sync.dma_start(out=idx2[:], in_=idx2_d[:])

            # ---- degrees -> deg_inv (from idx1 pad pattern, before gathers)
            deg_all = dpool.tile([P, nch], F32, tag="deg")
            for (k0, nk, colstart, dg) in groups:
                if dg == 0:
                    continue
                mt = mpool.tile([P, dg_tile], F32, tag="mask")
                nc.vector.tensor_scalar(
                    out=mt[:, :dg], in0=idx1[:, colstart : colstart + dg],
                    scalar1=dummy1, scalar2=None, op0=mybir.AluOpType.is_lt)
                for kk in range(nk):
                    k = k0 + kk
                    dk = int(D[k])
                    o = int(off[k]) - colstart
                    if dk == 0:
                        nc.vector.memset(deg_all[:, k : k + 1], 0.0)
                        continue
                    nc.vector.tensor_reduce(
                        out=deg_all[:, k : k + 1], in_=mt[:, o : o + dk],
                        axis=mybir.AxisListType.X, op=mybir.AluOpType.add)
            for (k0, nk, colstart, dg) in groups:
                if dg == 0:
                    for kk in range(nk):
                        nc.vector.memset(deg_all[:, k0 + kk : k0 + kk + 1], 0.0)
            dmax = dpool.tile([P, nch], F32, tag="dmax")
            nc.vector.tensor_scalar(
                out=dmax[:], in0=deg_all[:], scalar1=1.0, scalar2=None,
                op0=mybir.AluOpType.max)
            drec = dpool.tile([P, nch], F32, tag="drec")
            nc.vector.reciprocal(out=drec[:], in_=dmax[:])
            dnz = dpool.tile([P, nch], F32, tag="dnz")
            nc.vector.tensor_scalar(
                out=dnz[:], in0=deg_all[:], scalar1=0.0, scalar2=None,
                op0=mybir.AluOpType.is_gt)
            deginv = dpool.tile([P, nch], F32, tag="deginv")
            nc.vector.tensor_tensor(
                out=deginv[:], in0=drec[:], in1=dnz[:],
                op=mybir.AluOpType.mult)

            # ---- layer 1
            for (k0, nk, colstart, dg) in groups:
                gt = None
                if dg > 0:
                    gt = gpool.tile([P, dg_tile * f_in], F32, tag="gather")
                    for j in range(dg):
                        nc.gpsimd.indirect_dma_start(
                            out=gt[:, j * f_in : (j + 1) * f_in],
                            out_offset=None,
                            in_=feat_aug[:],
                            in_offset=bass.IndirectOffsetOnAxis(
                                ap=idx1[:, colstart + j : colstart + j + 1],
                                axis=0),
                        )
                ft = spool.tile([f_in, nk_tile * P], F32, tag="ftile")
                nc.sync.dma_start(
                    out=ft[:, : nk * P],
                    in_=featT[:, k0 * P : (k0 + nk) * P])
                for kk in range(nk):
                    k = k0 + kk
                    dk = int(D[k])
                    o = int(off[k]) - colstart
                    # segmented sum over this chunk's padded edges
                    hT_ps = psA.tile([fh, P], F32, tag="hT_ps")
                    nc.tensor.matmul(
                        out=hT_ps[:], lhsT=w1s[:],
                        rhs=ft[:, kk * P : (kk + 1) * P],
                        start=True, stop=(dk == 0))
                    if dk > 0:
                        agg = wpool.tile([P, f_in], F32, tag="agg")
                        gslice = gt[:, o * f_in : (o + dk) * f_in]
                        nc.vector.tensor_reduce(
                            out=agg[:],
                            in_=gslice.rearrange("p (j f) -> p f j", f=f_in),
                            axis=mybir.AxisListType.X, op=mybir.AluOpType.add)
                        mean = wpool.tile([P, f_in], F32, tag="mean")
                        nc.vector.tensor_scalar(
                            out=mean[:], in0=agg[:],
                            scalar1=deginv[:, k : k + 1], scalar2=None,
                            op0=mybir.AluOpType.mult)
                        mT_ps = psB.tile([f_in, P], F32, tag="mT_ps")
                        nc.tensor.transpose(
                            out=mT_ps[:], in_=mean[:], identity=ident[:])
                        mT = wpool.tile([f_in, P], F32, tag="mT")
                        nc.scalar.activation(
                            out=mT[:], in_=mT_ps[:],
                            func=mybir.ActivationFunctionType.Copy)
                        nc.tensor.matmul(
                            out=hT_ps[:], lhsT=w1n[:], rhs=mT[:],
                            start=False, stop=True)
                    hT = wpool.tile([fh, P], F32, tag="hT")
                    nc.scalar.activation(
                        out=hT[:], in_=hT_ps[:],
                        func=mybir.ActivationFunctionType.Relu,
                        bias=b1[:, :1])
                    nc.sync.dma_start(
                        out=hT_dram[:, k * P : (k + 1) * P], in_=hT[:])
                    p_ps = psB.tile([P, f_out], F32, tag="p_ps")
                    nc.tensor.matmul(
                        out=p_ps[:], lhsT=hT[:], rhs=w2n[:],
                        start=True, stop=True)
                    p_sb = wpool.tile([P, f_out], F32, tag="p_sb")
                    nc.scalar.activation(
                        out=p_sb[:], in_=p_ps[:],
                        func=mybir.ActivationFunctionType.Copy)
                    nc.sync.dma_start(
                        out=p_blk[k * P : (k + 1) * P, :], in_=p_sb[:])

            # ---- exchange p across cores
            nc.sync.dma_start(
                out=p_full[ncores * npad : ncores * npad + 1, :], in_=zrow[:])
            nc.gpsimd.collective_compute(
                "AllGather",
                mybir.AluOpType.bypass,
                replica_groups=[list(range(ncores))],
                ins=[p_blk[:]],
                outs=[p_full[: ncores * npad, :]],
            )

            # ---- layer 2
            for (k0, nk, colstart, dg) in groups:
                gt = None
                if dg > 0:
                    gt = gpool.tile([P, dg_tile * f_in], F32, tag="gather")
                    for j in range(dg):
                        nc.gpsimd.indirect_dma_start(
                            out=gt[:, j * f_out : (j + 1) * f_out],
                            out_offset=None,
                            in_=p_full[:],
                            in_offset=bass.IndirectOffsetOnAxis(
                                ap=idx2[:, colstart + j : colstart + j + 1],
                                axis=0),
                        )
                ht = spool.tile([fh, nk_tile * P], F32, tag="htile")
                nc.sync.dma_start(
                    out=ht[:, : nk * P],
                    in_=hT_dram[:, k0 * P : (k0 + nk) * P])
                for kk in range(nk):
                    k = k0 + kk
                    dk = int(D[k])
                    o = int(off[k]) - colstart
                    s_ps = psA.tile([P, f_out], F32, tag="s_ps")
                    nc.tensor.matmul(
                        out=s_ps[:], lhsT=ht[:, kk * P : (kk + 1) * P],
                        rhs=w2s[:], start=True, stop=True)
                    t_sb = wpool.tile([P, f_out], F32, tag="t_sb")
                    if dk > 0:
                        agg2 = wpool.tile([P, f_out], F32, tag="agg2")
                        gslice = gt[:, o * f_out : (o + dk) * f_out]
                        nc.vector.tensor_reduce(
                            out=agg2[:],
                            in_=gslice.rearrange("p (j f) -> p f j", f=f_out),
                            axis=mybir.AxisListType.X, op=mybir.AluOpType.add)
                        mean2 = wpool.tile([P, f_out], F32, tag="mean2")
                        nc.scalar.activation(
                            out=mean2[:], in_=agg2[:],
                            func=mybir.ActivationFunctionType.Copy,
                            scale=deginv[:, k : k + 1])
                        nc.vector.tensor_tensor(
                            out=t_sb[:], in0=s_ps[:], in1=mean2[:],
                            op=mybir.AluOpType.add)
                    else:
                        nc.vector.tensor_copy(out=t_sb[:], in_=s_ps[:])
                    # log_softmax over free dim
                    mx = smpool.tile([P, 1], F32, tag="mx")
                    nc.vector.tensor_reduce(
                        out=mx[:], in_=t_sb[:], axis=mybir.AxisListType.X,
                        op=mybir.AluOpType.max, negate=True)
                    ex = wpool.tile([P, f_out], F32, tag="ex")
                    se = smpool.tile([P, 1], F32, tag="se")
                    nc.scalar.activation(
                        out=ex[:], in_=t_sb[:],
                        func=mybir.ActivationFunctionType.Exp,
                        bias=mx[:, :1], accum_out=se[:, :1])
                    ln = smpool.tile([P, 1], F32, tag="ln")
                    nc.scalar.activation(
                        out=ln[:], in_=se[:],
                        func=mybir.ActivationFunctionType.Ln)
                    o_sb = wpool.tile([P, f_out], F32, tag="o_sb")
                    nc.vector.tensor_scalar(
                        out=o_sb[:], in0=t_sb[:],
                        scalar1=mx[:, :1], scalar2=ln[:, :1],
                        op0=mybir.AluOpType.add, op1=mybir.AluOpType.subtract)
                    nc.sync.dma_start(
                        out=out_d[k * P : (k + 1) * P, :], in_=o_sb[:])

    return nc


# --------------------------------------------------------------------------
# public entry
# --------------------------------------------------------------------------
def _run(feat, src, dst, W1_self, W1_neigh, b1, W2_self, W2_neigh, b2,
         ncores=NCORES, trace=False):
    global LAST_RESULTS
    n_nodes, f_in = feat.shape
    f_hid = W1_self.shape[1]
    f_out = W2_self.shape[1]
    fh = f_hid + 1

    src = np.asarray(src).astype(np.int64, copy=False)
    dst = np.asarray(dst).astype(np.int64, copy=False)
    feat = np.asarray(feat, dtype=np.float32)

    meta = _prep_indices(src, dst, n_nodes, ncores)
    groups = _make_groups(meta["D"], meta["off"])
    npad = meta["npad"]
    npc = meta["npc"]

    global LAST_NC
    nc = _build_program(meta, groups, f_in, f_hid, f_out, n_nodes, ncores)
    nc.compile()
    LAST_NC = nc

    # host-side input tensors
    feat_aug = np.zeros((n_nodes + 1, f_in), np.float32)
    feat_aug[:n_nodes] = feat
    w1s_aug = np.zeros((f_in, fh), np.float32)
    w1s_aug[:, :f_hid] = W1_self
    w1n_aug = np.zeros((f_in, fh), np.float32)
    w1n_aug[:, :f_hid] = W1_neigh
    b1_aug = np.zeros((fh, 1), np.float32)
    b1_aug[:f_hid, 0] = b1
    b1_aug[f_hid, 0] = 1.0
    w2s_aug = np.zeros((fh, f_out), np.float32)
    w2s_aug[:f_hid] = W2_self
    w2s_aug[f_hid] = b2
    w2n_aug = np.zeros((fh, f_out), np.float32)
    w2n_aug[:f_hid] = W2_neigh

    in_maps = []
    for c in range(ncores):
        gids = c * npc + meta["perms"][c]
        fT = np.zeros((f_in, npad), np.float32)
        fT[:, :npc] = feat[gids].T
        in_maps.append({
            "feat_aug": feat_aug,
            "featT": np.ascontiguousarray(fT),
            "idx1": meta["idx1"][c],
            "idx2": meta["idx2"][c],
            "w1s": w1s_aug,
            "w1n": w1n_aug,
            "b1a": b1_aug,
            "w2s": w2s_aug,
            "w2n": w2n_aug,
        })

    res = run_bass_kernel_spmd(nc, in_maps, list(range(ncores)), trace=trace)
    LAST_RESULTS = res

    out = np.empty((n_nodes, f_out), np.float32)
    for c in range(ncores):
        gids = c * npc + meta["perms"][c]
        out[gids] = res.results[c]["out_blk"][:npc]
    return out


def kernel(feat, src, dst, W1_self, W1_neigh, b1, W2_self, W2_neigh, b2):
    return _run(
        np.asarray(feat), np.asarray(src), np.asarray(dst),
        np.asarray(W1_self, dtype=np.float32),
        np.asarray(W1_neigh, dtype=np.float32),
        np.asarray(b1, dtype=np.float32),
        np.asarray(W2_self, dtype=np.float32),
        np.asarray(W2_neigh, dtype=np.float32),
        np.asarray(b2, dtype=np.float32),
        ncores=NCORES,
        trace=bool(int(os.environ.get("KERNEL_TRACE", "0"))),
    )
